# revision 18
# baseline (speedup 1.0000x reference)
"""3x3 conv (im2col formulation) as 9 shifted matmuls on TRN2, data-parallel over batch.

Full inputs: x [32, 128, 56, 56] f32, w [1152, 256] f32 (row = c*9 + kh*3 + kw).
Full output: [32, 256, 56, 56] f32.

Each of the 8 cores processes 4 batch images. Per core:
  - x image is DMA'd contiguously into a [128(c), 56, 56] staging tile
    (full-rate 12.5KB/partition descriptors), then copied on-chip into a
    zero-bordered [128, 58, 58] padded tile (f32r).
  - w is DMA'd once as [128(c), 9, 256] (f32r).
  - Per (image, out-channel half, 8-row band): 9 shifted matmuls accumulate
    w_tap.T @ x_shifted into a [128(o), 8, 56] PSUM bank (f32r = full PE
    rate, ~1e-4 rel err); DVE copies the band to a [128, 56, 56] SBUF image;
    each band streams to DRAM immediately (1792B/partition contiguous) on
    the scalar-engine HWDGE queue.
  - Dummy bf16 warmup matmuls run during the DMA lead-in to lift the PE HAM
    clock gate before the real stream starts.
"""

import numpy as np

import concourse.bass as bass  # noqa: F401  (registers AP types)
import concourse.mybir as mybir
import concourse.tile as tile
from concourse import bacc, bass_utils

B, C, H, W = 32, 128, 56, 56
COUT = 256
NCORES = 8
BPC = B // NCORES  # images per core
HP, WP = H + 2, W + 2
HROWS = 8  # output rows per PSUM band
HT = H // HROWS  # bands per image
F32 = mybir.dt.float32
F32R = mybir.dt.float32r
BF16 = mybir.dt.bfloat16

_cached_nc = None


def _build():
    nc = bacc.Bacc(None, target_bir_lowering=False)
    x = nc.dram_tensor("x", [BPC, C, H, W], F32, kind="ExternalInput")
    # host pre-arranges w as [oc_half, c, tap, 128] so each half DMAs with
    # fully contiguous per-partition chunks
    w = nc.dram_tensor("w", [2, C, 9, 128], F32, kind="ExternalInput")
    out = nc.dram_tensor("out", [BPC, COUT, H, W], F32, kind="ExternalOutput")

    with tile.TileContext(nc) as tc:
        with (
            tc.tile_pool(name="wpool", bufs=1) as wpool,
            tc.tile_pool(name="spool", bufs=2) as spool,
            tc.tile_pool(name="xpool", bufs=2) as xpool,
            tc.tile_pool(name="opool", bufs=2) as opool,
            tc.tile_pool(name="pspool", bufs=8, space="PSUM") as pspool,
        ):
            # PE warmup: tiny matmuls with no data deps keep the PE busy
            # during the input DMA so HAM reaches K=8/8 before the real work.
            # Full-width warmup keeps PE duty-cycle high enough to trip the
            # HAM activity monitor (N=16 warmups run at ~27% duty and don't).
            NWARM = 32
            warm = wpool.tile([C, 448], BF16)
            nc.vector.memset(warm[:], 0.0)
            wpsum = pspool.tile([16, 448], F32, tag="pt", name="warm_psum")
            for i in range(NWARM):
                nc.tensor.matmul(wpsum[:], warm[:, :16], warm[:],
                                 start=(i == 0), stop=(i == NWARM - 1))

            # Input bandwidth is shared (~330GB/s), so sequence the sync-ring
            # DMAs to put the minimum bytes ahead of the first matmul:
            # xs0-lower, w-oc0, xs0-upper, w-oc1, then the remaining images.
            # The image-0 pad copy is split so bands 0-2 start after the
            # lower half lands.
            HSPL = 28
            wbuf = wpool.tile([C, 2, 9, 128], F32R)
            xs0 = spool.tile([C, H, W], F32R, tag="xs", name="xs0")
            nc.sync.dma_start(xs0[:, :HSPL, :], x[0, :, :HSPL, :].bitcast(F32R))
            nc.sync.dma_start(wbuf[:, 0], w[0].bitcast(F32R))
            nc.sync.dma_start(xs0[:, HSPL:, :], x[0, :, HSPL:, :].bitcast(F32R))
            nc.sync.dma_start(wbuf[:, 1], w[1].bitcast(F32R))

            for b in range(BPC):
                if b == 0:
                    xs = xs0
                else:
                    xs = spool.tile([C, H, W], F32R, tag="xs", name=f"xs{b}")
                    nc.sync.dma_start(xs[:], x[b].bitcast(F32R))

                xp = xpool.tile([C, HP, WP], F32R, tag="xp", name=f"xp{b}")
                nc.vector.memset(xp[:, 0, :].bitcast(F32), 0.0)
                nc.vector.memset(xp[:, HP - 1, :].bitcast(F32), 0.0)
                nc.vector.memset(xp[:, :, 0].bitcast(F32), 0.0)
                nc.vector.memset(xp[:, :, WP - 1].bitcast(F32), 0.0)
                if b == 0:
                    nc.vector.tensor_copy(
                        out=xp[:, 1 : HSPL + 1, 1 : W + 1], in_=xs[:, :HSPL, :]
                    )
                    nc.vector.tensor_copy(
                        out=xp[:, HSPL + 1 : H + 1, 1 : W + 1], in_=xs[:, HSPL:, :]
                    )
                else:
                    nc.vector.tensor_copy(out=xp[:, 1 : H + 1, 1 : W + 1], in_=xs[:])

                for oc in range(COUT // 128):
                    oimg = opool.tile([128, H, W], F32, tag="oimg", name=f"oimg{b}_{oc}")
                    for ht in range(HT):
                        pt = pspool.tile(
                            [128, HROWS, W], F32, tag="pt", name=f"pt{b}_{oc}_{ht}"
                        )
                        for dh in (-1, 0, 1):
                            for dw in (-1, 0, 1):
                                kk = (dh + 1) * 3 + (dw + 1)
                                h0 = ht * HROWS + dh + 1
                                rhs = xp[:, h0 : h0 + HROWS, dw + 1 : dw + 1 + W]
                                lhsT = wbuf[:, oc, kk, :]
                                nc.tensor.matmul(
                                    pt[:], lhsT, rhs, start=(kk == 0), stop=(kk == 8)
                                )
                        last_band = b == BPC - 1 and oc == 1 and ht == HT - 1
                        if last_band:
                            # split the final band so only a half-band of
                            # copy+DMA is exposed after the last matmul
                            hh = HROWS // 2
                            for part in range(2):
                                r0 = ht * HROWS + part * hh
                                nc.vector.tensor_copy(
                                    out=oimg[:, r0 : r0 + hh, :],
                                    in_=pt[:, part * hh : (part + 1) * hh, :],
                                )
                                nc.scalar.dma_start(
                                    out[b, oc * 128 : (oc + 1) * 128, r0 : r0 + hh, :],
                                    oimg[:, r0 : r0 + hh, :],
                                )
                        else:
                            nc.vector.tensor_copy(
                                out=oimg[:, ht * HROWS : (ht + 1) * HROWS, :], in_=pt[:]
                            )
                            nc.scalar.dma_start(
                                out[b, oc * 128 : (oc + 1) * 128,
                                    ht * HROWS : (ht + 1) * HROWS, :],
                                oimg[:, ht * HROWS : (ht + 1) * HROWS, :],
                            )
    nc.compile()
    return nc


def _get_nc():
    global _cached_nc
    if _cached_nc is None:
        _cached_nc = _build()
    return _cached_nc


def run(x, w, trace=False, **spmd_kwargs):
    nc = _get_nc()
    x = np.ascontiguousarray(x, dtype=np.float32)
    w = np.asarray(w, dtype=np.float32)
    # [c*9, 256] -> [oc_half, c, tap, 128]
    w2 = np.ascontiguousarray(
        w.reshape(C, 9, 2, 128).transpose(2, 0, 1, 3)
    )
    in_maps = [
        {"x": x[i * BPC : (i + 1) * BPC], "w": w2} for i in range(NCORES)
    ]
    res = bass_utils.run_bass_kernel_spmd(
        nc, in_maps, core_ids=list(range(NCORES)), trace=trace, **spmd_kwargs
    )
    full = np.concatenate([r["out"] for r in res.results], axis=0)
    return full, res


def kernel(x, w):
    return run(x, w)[0]


# revision 19
# speedup vs baseline: 1.1137x; 1.1137x over previous
"""3x3 conv (im2col formulation) as 9 shifted matmuls on TRN2, data-parallel over batch.

Full inputs: x [32, 128, 56, 56] f32, w [1152, 256] f32 (row = c*9 + kh*3 + kw).
Full output: [32, 256, 56, 56] f32.

Each of the 8 cores processes 4 batch images. Per core:
  - x image is DMA'd contiguously into a [128(c), 56, 56] staging tile
    (full-rate 12.5KB/partition descriptors), then copied on-chip into a
    zero-bordered [128, 58, 58] padded tile (f32r).
  - w is DMA'd once as [128(c), 9, 256] (f32r).
  - Per (image, out-channel half, 8-row band): 9 shifted matmuls accumulate
    w_tap.T @ x_shifted into a [128(o), 8, 56] PSUM bank (f32r = full PE
    rate, ~1e-4 rel err); DVE copies the band to a [128, 56, 56] SBUF image;
    each band streams to DRAM immediately (1792B/partition contiguous) on
    the scalar-engine HWDGE queue.
  - Dummy bf16 warmup matmuls run during the DMA lead-in to lift the PE HAM
    clock gate before the real stream starts.
"""

import numpy as np

import concourse.bass as bass  # noqa: F401  (registers AP types)
import concourse.mybir as mybir
import concourse.tile as tile
from concourse import bacc, bass_utils

B, C, H, W = 32, 128, 56, 56
COUT = 256
NCORES = 8
BPC = B // NCORES  # images per core
HP, WP = H + 2, W + 2
HROWS = 8  # output rows per PSUM band
HT = H // HROWS  # bands per image
F32 = mybir.dt.float32
F32R = mybir.dt.float32r
BF16 = mybir.dt.bfloat16
MOV = mybir.dt.float16  # matmul operand dtype (fp16: full PE rate, FWL LDW)
MOV_NP = np.float16

_cached_nc = None


def _build():
    nc = bacc.Bacc(None, target_bir_lowering=False)
    x = nc.dram_tensor("x", [BPC, C, H, W], MOV, kind="ExternalInput")
    # host pre-arranges w as [oc_half, c, tap, 128] so each half DMAs with
    # fully contiguous per-partition chunks
    w = nc.dram_tensor("w", [2, C, 9, 128], MOV, kind="ExternalInput")
    out = nc.dram_tensor("out", [BPC, COUT, H, W], F32, kind="ExternalOutput")

    with tile.TileContext(nc) as tc:
        with (
            tc.tile_pool(name="wpool", bufs=1) as wpool,
            tc.tile_pool(name="spool", bufs=2) as spool,
            tc.tile_pool(name="xpool", bufs=2) as xpool,
            tc.tile_pool(name="opool", bufs=2) as opool,
            tc.tile_pool(name="pspool", bufs=8, space="PSUM") as pspool,
        ):
            # PE warmup: tiny matmuls with no data deps keep the PE busy
            # during the input DMA so HAM reaches K=8/8 before the real work.
            # Full-width warmup keeps PE duty-cycle high enough to trip the
            # HAM activity monitor (N=16 warmups run at ~27% duty and don't).
            NWARM = 13
            warm = wpool.tile([C, 448], BF16)
            nc.vector.memset(warm[:], 0.0)
            wpsum = pspool.tile([16, 448], F32, tag="pt", name="warm_psum")
            for i in range(NWARM):
                nc.tensor.matmul(wpsum[:], warm[:, :16], warm[:],
                                 start=(i == 0), stop=(i == NWARM - 1))

            # Input bandwidth is shared (~330GB/s), so sequence the sync-ring
            # DMAs to put the minimum bytes ahead of the first matmul:
            # xs0-lower, w-oc0, xs0-upper, w-oc1, then the remaining images.
            # The image-0 pad copy is split so bands 0-2 start after the
            # lower half lands.
            HSPL = 28
            wbuf = wpool.tile([C, 2, 9, 128], MOV)
            xs0 = spool.tile([C, H, W], MOV, tag="xs", name="xs0")
            nc.sync.dma_start(xs0[:, :HSPL, :], x[0, :, :HSPL, :])
            nc.sync.dma_start(wbuf[:, 0], w[0])
            nc.sync.dma_start(xs0[:, HSPL:, :], x[0, :, HSPL:, :])
            nc.sync.dma_start(wbuf[:, 1], w[1])

            for b in range(BPC):
                if b == 0:
                    xs = xs0
                else:
                    xs = spool.tile([C, H, W], MOV, tag="xs", name=f"xs{b}")
                    nc.sync.dma_start(xs[:], x[b])

                xp = xpool.tile([C, HP, WP], MOV, tag="xp", name=f"xp{b}")
                nc.vector.memset(xp[:, 0, :], 0.0)
                nc.vector.memset(xp[:, HP - 1, :], 0.0)
                nc.vector.memset(xp[:, :, 0], 0.0)
                nc.vector.memset(xp[:, :, WP - 1], 0.0)
                if b == 0:
                    nc.vector.tensor_copy(
                        out=xp[:, 1 : HSPL + 1, 1 : W + 1], in_=xs[:, :HSPL, :]
                    )
                    nc.vector.tensor_copy(
                        out=xp[:, HSPL + 1 : H + 1, 1 : W + 1], in_=xs[:, HSPL:, :]
                    )
                else:
                    nc.vector.tensor_copy(out=xp[:, 1 : H + 1, 1 : W + 1], in_=xs[:])

                for oc in range(COUT // 128):
                    oimg = opool.tile([128, H, W], F32, tag="oimg", name=f"oimg{b}_{oc}")
                    for ht in range(HT):
                        pt = pspool.tile(
                            [128, HROWS, W], F32, tag="pt", name=f"pt{b}_{oc}_{ht}"
                        )
                        for dh in (-1, 0, 1):
                            for dw in (-1, 0, 1):
                                kk = (dh + 1) * 3 + (dw + 1)
                                h0 = ht * HROWS + dh + 1
                                rhs = xp[:, h0 : h0 + HROWS, dw + 1 : dw + 1 + W]
                                lhsT = wbuf[:, oc, kk, :]
                                nc.tensor.matmul(
                                    pt[:], lhsT, rhs, start=(kk == 0), stop=(kk == 8)
                                )
                        last_band = b == BPC - 1 and oc == 1 and ht == HT - 1
                        if last_band:
                            # split the final band so only a half-band of
                            # copy+DMA is exposed after the last matmul
                            hh = HROWS // 2
                            for part in range(2):
                                r0 = ht * HROWS + part * hh
                                nc.vector.tensor_copy(
                                    out=oimg[:, r0 : r0 + hh, :],
                                    in_=pt[:, part * hh : (part + 1) * hh, :],
                                )
                                nc.scalar.dma_start(
                                    out[b, oc * 128 : (oc + 1) * 128, r0 : r0 + hh, :],
                                    oimg[:, r0 : r0 + hh, :],
                                )
                        else:
                            nc.vector.tensor_copy(
                                out=oimg[:, ht * HROWS : (ht + 1) * HROWS, :], in_=pt[:]
                            )
                            nc.scalar.dma_start(
                                out[b, oc * 128 : (oc + 1) * 128,
                                    ht * HROWS : (ht + 1) * HROWS, :],
                                oimg[:, ht * HROWS : (ht + 1) * HROWS, :],
                            )
    nc.compile()
    return nc


def _get_nc():
    global _cached_nc
    if _cached_nc is None:
        _cached_nc = _build()
    return _cached_nc


def run(x, w, trace=False, **spmd_kwargs):
    nc = _get_nc()
    x = np.ascontiguousarray(x, dtype=np.float32).astype(MOV_NP)
    w = np.asarray(w, dtype=np.float32)
    # [c*9, 256] -> [oc_half, c, tap, 128]
    w2 = np.ascontiguousarray(
        w.reshape(C, 9, 2, 128).transpose(2, 0, 1, 3)
    ).astype(MOV_NP)
    in_maps = [
        {"x": x[i * BPC : (i + 1) * BPC], "w": w2} for i in range(NCORES)
    ]
    res = bass_utils.run_bass_kernel_spmd(
        nc, in_maps, core_ids=list(range(NCORES)), trace=trace, **spmd_kwargs
    )
    full = np.concatenate([r["out"] for r in res.results], axis=0)
    return full, res


def kernel(x, w):
    return run(x, w)[0]


# revision 20
# speedup vs baseline: 1.1227x; 1.0080x over previous
"""3x3 conv (im2col formulation) as 9 shifted matmuls on TRN2, data-parallel over batch.

Full inputs: x [32, 128, 56, 56] f32, w [1152, 256] f32 (row = c*9 + kh*3 + kw).
Full output: [32, 256, 56, 56] f32.

Each of the 8 cores processes 4 batch images. Per core:
  - x image is DMA'd contiguously into a [128(c), 56, 56] staging tile
    (full-rate 12.5KB/partition descriptors), then copied on-chip into a
    zero-bordered [128, 58, 58] padded tile (f32r).
  - w is DMA'd once as [128(c), 9, 256] (f32r).
  - Per (image, out-channel half, 8-row band): 9 shifted matmuls accumulate
    w_tap.T @ x_shifted into a [128(o), 8, 56] PSUM bank (f32r = full PE
    rate, ~1e-4 rel err); DVE copies the band to a [128, 56, 56] SBUF image;
    each band streams to DRAM immediately (1792B/partition contiguous) on
    the scalar-engine HWDGE queue.
  - Dummy bf16 warmup matmuls run during the DMA lead-in to lift the PE HAM
    clock gate before the real stream starts.
"""

import numpy as np

import concourse.bass as bass  # noqa: F401  (registers AP types)
import concourse.mybir as mybir
import concourse.tile as tile
from concourse import bacc, bass_utils

B, C, H, W = 32, 128, 56, 56
COUT = 256
NCORES = 8
BPC = B // NCORES  # images per core
HP = H + 2
# tap order: dw=0 taps first (full width, carries the PSUM start flag)
TAPS = ([(dh, 0) for dh in (-1, 0, 1)]
        + [(dh, -1) for dh in (-1, 0, 1)]
        + [(dh, 1) for dh in (-1, 0, 1)])
HROWS = 8  # output rows per PSUM band
HT = H // HROWS  # bands per image
F32 = mybir.dt.float32
F32R = mybir.dt.float32r
BF16 = mybir.dt.bfloat16
MOV = mybir.dt.float16  # matmul operand dtype (fp16: full PE rate, FWL LDW)
MOV_NP = np.float16

_cached_nc = None


def _build():
    nc = bacc.Bacc(None, target_bir_lowering=False)
    x = nc.dram_tensor("x", [BPC, C, H, W], MOV, kind="ExternalInput")
    # host pre-arranges w as [oc_half, c, tap, 128] so each half DMAs with
    # fully contiguous per-partition chunks
    w = nc.dram_tensor("w", [2, C, 9, 128], MOV, kind="ExternalInput")
    out = nc.dram_tensor("out", [BPC, COUT, H, W], F32, kind="ExternalOutput")

    with tile.TileContext(nc) as tc:
        with (
            tc.tile_pool(name="wpool", bufs=1) as wpool,
            tc.tile_pool(name="xpool", bufs=2) as xpool,
            tc.tile_pool(name="opool", bufs=2) as opool,
            tc.tile_pool(name="pspool", bufs=8, space="PSUM") as pspool,
        ):
            # PE warmup: tiny matmuls with no data deps keep the PE busy
            # during the input DMA so HAM reaches K=8/8 before the real work.
            # Full-width warmup keeps PE duty-cycle high enough to trip the
            # HAM activity monitor (N=16 warmups run at ~27% duty and don't).
            NWARM = 12
            warm = wpool.tile([C, 448], BF16)
            nc.vector.memset(warm[:], 0.0)
            wpsum = pspool.tile([16, 448], F32, tag="pt", name="warm_psum")
            for i in range(NWARM):
                nc.tensor.matmul(wpsum[:], warm[:, :16], warm[:],
                                 start=(i == 0), stop=(i == NWARM - 1))

            # h-padded only ([C, 58, 56]): the input DMA destination is
            # fully contiguous per partition, so images load straight into
            # the compute tile — no staging, no pad copy. Horizontal taps
            # use 55-wide matmuls into offset PSUM slices instead.
            # Image 0 is split so bands 0-2 start after the lower half.
            HSPL = 28
            wbuf = wpool.tile([C, 2, 9, 128], MOV)
            xp0 = xpool.tile([C, HP, W], MOV, tag="xp", name="xp0")
            nc.sync.dma_start(xp0[:, 1 : HSPL + 1, :], x[0, :, :HSPL, :])
            nc.sync.dma_start(wbuf[:, 0], w[0])
            nc.sync.dma_start(xp0[:, HSPL + 1 : H + 1, :], x[0, :, HSPL:, :])
            nc.sync.dma_start(wbuf[:, 1], w[1])

            for b in range(BPC):
                if b == 0:
                    xp = xp0
                else:
                    xp = xpool.tile([C, HP, W], MOV, tag="xp", name=f"xp{b}")
                    nc.sync.dma_start(xp[:, 1 : H + 1, :], x[b])
                nc.vector.memset(xp[:, 0, :], 0.0)
                nc.vector.memset(xp[:, HP - 1, :], 0.0)

                for oc in range(COUT // 128):
                    oimg = opool.tile([128, H, W], F32, tag="oimg", name=f"oimg{b}_{oc}")
                    for ht in range(HT):
                        pt = pspool.tile(
                            [128, HROWS, W], F32, tag="pt", name=f"pt{b}_{oc}_{ht}"
                        )
                        for t, (dh, dw) in enumerate(TAPS):
                            kk = (dh + 1) * 3 + (dw + 1)
                            h0 = ht * HROWS + dh + 1
                            if dw == 0:
                                rhs = xp[:, h0 : h0 + HROWS, :]
                                dst = pt[:]
                            elif dw == -1:
                                rhs = xp[:, h0 : h0 + HROWS, 0 : W - 1]
                                dst = pt[:, :, 1:W]
                            else:
                                rhs = xp[:, h0 : h0 + HROWS, 1:W]
                                dst = pt[:, :, 0 : W - 1]
                            lhsT = wbuf[:, oc, kk, :]
                            nc.tensor.matmul(
                                dst, lhsT, rhs, start=(t == 0), stop=(t == 8)
                            )
                        last_band = b == BPC - 1 and oc == 1 and ht == HT - 1
                        if last_band:
                            # split the final band so only a half-band of
                            # copy+DMA is exposed after the last matmul
                            hh = HROWS // 2
                            for part in range(2):
                                r0 = ht * HROWS + part * hh
                                nc.vector.tensor_copy(
                                    out=oimg[:, r0 : r0 + hh, :],
                                    in_=pt[:, part * hh : (part + 1) * hh, :],
                                )
                                nc.scalar.dma_start(
                                    out[b, oc * 128 : (oc + 1) * 128, r0 : r0 + hh, :],
                                    oimg[:, r0 : r0 + hh, :],
                                )
                        else:
                            nc.vector.tensor_copy(
                                out=oimg[:, ht * HROWS : (ht + 1) * HROWS, :], in_=pt[:]
                            )
                            nc.scalar.dma_start(
                                out[b, oc * 128 : (oc + 1) * 128,
                                    ht * HROWS : (ht + 1) * HROWS, :],
                                oimg[:, ht * HROWS : (ht + 1) * HROWS, :],
                            )
    nc.compile()
    return nc


def _get_nc():
    global _cached_nc
    if _cached_nc is None:
        _cached_nc = _build()
    return _cached_nc


def run(x, w, trace=False, **spmd_kwargs):
    nc = _get_nc()
    x = np.ascontiguousarray(x, dtype=np.float32).astype(MOV_NP)
    w = np.asarray(w, dtype=np.float32)
    # [c*9, 256] -> [oc_half, c, tap, 128]
    w2 = np.ascontiguousarray(
        w.reshape(C, 9, 2, 128).transpose(2, 0, 1, 3)
    ).astype(MOV_NP)
    in_maps = [
        {"x": x[i * BPC : (i + 1) * BPC], "w": w2} for i in range(NCORES)
    ]
    res = bass_utils.run_bass_kernel_spmd(
        nc, in_maps, core_ids=list(range(NCORES)), trace=trace, **spmd_kwargs
    )
    full = np.concatenate([r["out"] for r in res.results], axis=0)
    return full, res


def kernel(x, w):
    return run(x, w)[0]
